# revision 20
# baseline (speedup 1.0000x reference)
"""F1-score (macro) kernel for Trainium2, 8 NeuronCores.

Per core (data-parallel over rows), tiles of TK*128 rows ([128p, TK, 128c],
row = base + p*TK + k):
  - DVE:  rowmax via tensor_reduce (X axis, split in halves)
  - ACT:  anti-one-hot(pred) = sign(rowmax - x) in {0,1}  (most chunks)
  - GS :  a couple of anti chunks via is_lt
  - DVE:  one-hot(true) = (iota == y_true) via broadcast-TT ranges
  - PE :  cm_dev += one_hot_trueT @ anti  (bf16 matmuls, fp32 PSUM)
Host: cm = support[t] - sum_cores(cm_dev);  macro-F1 epilogue on [128,128].
All comparisons in exact fp32 -> bit-exact confusion matrix.
"""

import sys
import time

if "/opt/trn_rl_repo" not in sys.path:
    sys.path.insert(0, "/opt/trn_rl_repo")

import numpy as np

import concourse.bacc as bacc
import concourse.mybir as mybir
import concourse.tile as tile
from concourse import bass_utils

C = 128
N = 1_000_000
NCORES = 8
R = N // NCORES          # 125000 rows per core
TK = 16                  # chunks (of 128 rows) per big tile
TR = 128 * TK            # 4096 rows per big tile
NT = R // TR             # 30 big tiles
MID = (R - NT * TR) // 128   # leftover full chunks (16)
TAIL = R - NT * TR - MID * 128  # 72 rows
EPS = 1e-12

N_GS = 1                 # trailing anti chunks per big tile on GpSimd

_CACHE = {}


def _build():
    f32 = mybir.dt.float32
    bf16 = mybir.dt.bfloat16
    Alu = mybir.AluOpType
    Act = mybir.ActivationFunctionType

    nc = bacc.Bacc("TRN2", target_bir_lowering=False, debug=False,
                   num_devices=NCORES)
    yp = nc.dram_tensor("yp", [R, C], f32, kind="ExternalInput")
    yt = nc.dram_tensor("yt", [R], f32, kind="ExternalInput")
    cm = nc.dram_tensor("cm", [C, C], f32, kind="ExternalOutput")

    with tile.TileContext(nc) as tc:
        with (
            tc.tile_pool(name="const", bufs=1) as cpool,
            tc.tile_pool(name="xin", bufs=5) as xpool,
            tc.tile_pool(name="oh", bufs=6) as ohpool,
            tc.tile_pool(name="small", bufs=6) as spool,
            tc.tile_pool(name="psum", bufs=1, space="PSUM") as psum,
        ):
            iota_i = cpool.tile([128, C], mybir.dt.int32)
            nc.gpsimd.iota(iota_i[:], pattern=[[1, C]], base=0,
                           channel_multiplier=0)
            iota_bf = cpool.tile([128, C], bf16)
            nc.vector.tensor_copy(iota_bf[:], iota_i[:])
            iota_rep = cpool.tile([128, TK, C], bf16)
            nc.vector.tensor_copy(
                iota_rep[:], iota_bf[:, None, :].broadcast_to([128, TK, C])
            )

            acc = psum.tile([C, C], f32)
            state = {"started": False}

            def emit_tile(base, tk, n_gs):
                """One tile of 128*tk rows at row `base`: row = base+p*tk+k."""
                x = xpool.tile([128, tk, C], f32, tag="x")
                nc.sync.dma_start(
                    x[:],
                    yp.ap()[base : base + 128 * tk, :].rearrange(
                        "(p k) c -> p k c", k=tk
                    ),
                )
                t = spool.tile([128, tk], f32, tag="t")
                nc.sync.dma_start(
                    t[:],
                    yt.ap()[base : base + 128 * tk].rearrange(
                        "(p k) -> p k", k=tk
                    ),
                )
                rmax = spool.tile([128, tk], f32, tag="rmax")
                anti = ohpool.tile([128, tk, C], bf16, tag="anti")
                oht = ohpool.tile([128, tk, C], bf16, tag="oht")
                # oht first: depends only on the small y_true DMA, so DVE can
                # build it while the 1MB x DMA is still streaming in.
                for lo in range(0, tk, 8):
                    hi = min(lo + 8, tk)
                    w = hi - lo
                    nc.vector.tensor_tensor(
                        oht[:, lo:hi, :], iota_rep[:, lo:hi, :],
                        t[:, lo:hi, None].broadcast_to([128, w, C]),
                        op=Alu.is_equal,
                    )
                h = tk // 2
                for lo, hi in ((0, h), (h, tk)):
                    nc.vector.tensor_reduce(
                        rmax[:, lo:hi], x[:, lo:hi, :],
                        axis=mybir.AxisListType.X, op=Alu.max,
                    )
                for k in range(tk - n_gs):
                    nc.scalar.activation(
                        anti[:, k, :], x[:, k, :], Act.Sign,
                        bias=rmax[:, k : k + 1], scale=-1.0,
                    )
                for k in range(tk - n_gs, tk):
                    nc.gpsimd.tensor_scalar(
                        anti[:, k, :], x[:, k, :], rmax[:, k : k + 1], None,
                        op0=Alu.is_lt,
                    )
                for k in range(tk):
                    nc.tensor.matmul(
                        acc[:], oht[:, k, :], anti[:, k, :],
                        start=not state["started"], stop=False,
                    )
                    state["started"] = True

            for i in range(NT):
                emit_tile(i * TR, TK, N_GS)
            if MID:
                emit_tile(NT * TR, MID, 1)

            # tail rows (72), all on DVE
            base = NT * TR + MID * 128
            xt = xpool.tile([TAIL, 1, C], f32, tag="xtail")
            nc.sync.dma_start(
                xt[:],
                yp.ap()[base : R, :].rearrange("(p k) c -> p k c", k=1),
            )
            tt = spool.tile([TAIL, 1], f32, tag="ttail")
            nc.sync.dma_start(
                tt[:], yt.ap()[base : R].rearrange("(p k) -> p k", k=1)
            )
            rmax_t = spool.tile([TAIL, 1], f32, tag="rmaxtail")
            nc.vector.tensor_reduce(
                rmax_t[:], xt[:], axis=mybir.AxisListType.X, op=Alu.max
            )
            anti_t = ohpool.tile([TAIL, C], bf16, tag="antitail")
            oht_t = ohpool.tile([TAIL, C], bf16, tag="ohttail")
            nc.vector.tensor_scalar(
                anti_t[:], xt[:, 0, :], rmax_t[:], None, op0=Alu.is_lt
            )
            nc.vector.tensor_scalar(
                oht_t[:], iota_bf[:TAIL, :], tt[:], None, op0=Alu.is_equal
            )
            nc.tensor.matmul(
                acc[:], oht_t[:], anti_t[:], start=False, stop=True
            )

            out_sb = spool.tile([C, C], f32, tag="out")
            nc.scalar.copy(out_sb[:], acc[:])
            nc.sync.dma_start(cm.ap()[:], out_sb[:])

    nc.compile()
    return nc


def _get_nc():
    if "nc" not in _CACHE:
        _CACHE["nc"] = _build()
    return _CACHE["nc"]


def _run(y_pred, y_true, trace=False):
    nc = _get_nc()
    y_pred = np.ascontiguousarray(np.asarray(y_pred, dtype=np.float32))
    yt_i = np.asarray(y_true).astype(np.int64)
    yt_f = yt_i.astype(np.float32)
    in_maps = [
        {
            "yp": y_pred[c * R : (c + 1) * R],
            "yt": np.ascontiguousarray(yt_f[c * R : (c + 1) * R]),
        }
        for c in range(NCORES)
    ]
    res = None
    for attempt in range(3):
        try:
            res = bass_utils.run_bass_kernel_spmd(
                nc, in_maps, core_ids=list(range(NCORES)), trace=trace
            )
            break
        except Exception:
            if attempt == 2:
                raise
            time.sleep(2.0)
    cm_dev = np.zeros((C, C), dtype=np.float64)
    for r in res.results:
        cm_dev += r["cm"].astype(np.float64)
    support = np.bincount(yt_i, minlength=C).astype(np.float64)
    cm = support[:, None] - cm_dev
    diag = np.diagonal(cm)
    precision = diag / (cm.sum(axis=1) + EPS)
    recall = diag / (cm.sum(axis=0) + EPS)
    f1 = 2.0 * precision * recall / (precision + recall + EPS)
    return np.float32(f1.mean()), res


def kernel(y_pred, y_true):
    out, _ = _run(y_pred, y_true, trace=False)
    return out
